# revision 13
# baseline (speedup 1.0000x reference)
"""Trainium2 Bass kernel for the channel-attention module.

Reference computation (B=16, N=4096, C=384, H=8, D=48):
    x_in = x @ conv_w.T + conv_b                     # 1x1 conv == linear
    q    = (x_in @ wq.T + bq)  -> [B,H,D,N]
    k, v = (x_in @ wkv.T + bkv) -> 2x [B,H,D,N]
    attn = softmax((q * N**-0.5) @ k^T, axis=-1)      # [B,H,D,D] (over N!)
    out  = attn @ v                                   # [B,H,D,N]
    out  = out.transpose(0,2,1,3).reshape(B,N,C)      # verbatim torch layout
    y    = out @ wp.T + bp

Strategy: pure data parallelism over B across 8 NeuronCores (2 batches per
core), no collectives.  The conv is folded into the q/k/v projections on the
host (w_eff = (w @ conv_w).T, b_eff = b + w @ conv_b), so the device computes
q/k/v straight from x.  All big matmuls run as float32r (full-rate fp32 on
the PE at free-dim >= 256); the tiny per-head S = q^T k matmuls run in bf16.

The awkward transpose(0,2,1,3).reshape is handled exactly with 128-element
flat blocks: flat index (di, h, n) -> block u = 256*di + 32*h + n//128 with
intra-block offset c' = n%128.  Stage 5 (attn @ v) produces AT[c', u] tiles
directly (u = 32*(8*di+h) + t per n-chunk t), and stage 6 reads columns
u = 3*r + j (stride-3 APs) as the K=128 slices of the final projection.
"""

import sys
import types
from contextlib import ExitStack

import numpy as np

import concourse.bass as bass
import concourse.tile as tile
from concourse import bacc, mybir
from concourse.bass_utils import run_bass_kernel_spmd
from concourse.masks import make_identity

B, N, C, H, D = 16, 4096, 384, 8, 48
N_CORES = 8
BPC = B // N_CORES          # batches per core
NW = 512                    # token window for projection matmuls
NWIN = N // NW              # 8 windows
NCHUNK = N // 128           # 32 token chunks of 128
SCALE = float(N) ** -0.5    # 1/64
F32 = mybir.dt.float32
F32R = mybir.dt.float32r
BF16 = mybir.dt.bfloat16


def _install_ntff_hook():
    """The agent image's antenv lacks axon_hooks, so trn_boot's NTFF hook
    registration degrades silently and trace=True would crash.  Recreate the
    module and register the ctypes hook so profiling works."""
    try:
        import antenv

        if "antenv.axon_hooks" in sys.modules:
            return
        mod = types.ModuleType("antenv.axon_hooks")
        mod._hook = None
        mod.set_axon_ntff_profile_hook = lambda h: setattr(mod, "_hook", h)
        mod.get_axon_ntff_profile_hook = lambda: mod._hook
        sys.modules["antenv.axon_hooks"] = mod
        antenv.axon_hooks = mod
        from trn_agent_boot.trn_boot import _ntff_profile_via_ctypes

        mod.set_axon_ntff_profile_hook(
            _ntff_profile_via_ctypes("/opt/axon/libaxon_pjrt.so")
        )
    except Exception:
        pass


def build():
    nc = bacc.Bacc("TRN2", target_bir_lowering=False, debug=False,
                   num_devices=N_CORES)

    # Per-core inputs.  x is pre-transposed on the host to [BPC, C, N].
    xp = nc.declare_dram_parameter("x", [BPC, C, N], F32R, isOutput=False)
    wq_p = nc.declare_dram_parameter("wqT", [C, C], F32R, isOutput=False)
    wk_p = nc.declare_dram_parameter("wkT", [C, C], F32R, isOutput=False)
    wv_p = nc.declare_dram_parameter("wvT", [C, C], F32R, isOutput=False)
    wp_p = nc.declare_dram_parameter("wpT", [C, C], F32R, isOutput=False)
    bq_p = nc.declare_dram_parameter("bq", [C], F32, isOutput=False)
    bk_p = nc.declare_dram_parameter("bk", [C], F32, isOutput=False)
    bv_p = nc.declare_dram_parameter("bv", [C], F32, isOutput=False)
    bp_p = nc.declare_dram_parameter("bp", [C], F32, isOutput=False)
    z_p = nc.declare_dram_parameter("zeros", [128, C], F32R, isOutput=False)
    outp = nc.declare_dram_parameter("out", [BPC, N, C], F32, isOutput=True)

    with tile.TileContext(nc) as tc, ExitStack() as ctx:
        const = ctx.enter_context(tc.tile_pool(name="const", bufs=1))
        xin = ctx.enter_context(tc.tile_pool(name="xin", bufs=3))
        qk = ctx.enter_context(tc.tile_pool(name="qk", bufs=6))
        big = ctx.enter_context(tc.tile_pool(name="big", bufs=1))
        sm = ctx.enter_context(tc.tile_pool(name="sm", bufs=2))
        yout = ctx.enter_context(tc.tile_pool(name="yout", bufs=3))
        ps_big = ctx.enter_context(tc.tile_pool(name="ps_big", bufs=2, space="PSUM"))
        ps_qk = ctx.enter_context(tc.tile_pool(name="ps_qk", bufs=3, space="PSUM"))
        ps_s = ctx.enter_context(tc.tile_pool(name="ps_s", bufs=1, space="PSUM"))
        ps_tr = ctx.enter_context(tc.tile_pool(name="ps_tr", bufs=1, space="PSUM"))

        # ---- constants -----------------------------------------------------
        def load_w(param):
            t = const.tile([128, 3, C], F32R, tag=f"w_{param.name}")
            nc.sync.dma_start(t[:], param.ap().rearrange("(kc p) o -> p kc o", p=128))
            return t

        wq_sb, wk_sb, wv_sb, wp_sb = (load_w(p) for p in (wq_p, wk_p, wv_p, wp_p))

        def load_bias_bcast(param):
            # replicate a [C] bias across all 128 partitions
            t = const.tile([128, C], F32, tag=f"bb_{param.name}")
            ap = param.ap()
            src = bass.AP(tensor=ap.tensor, offset=ap.offset,
                          ap=[[0, 128], *ap.ap])
            nc.sync.dma_start(t[:], src)
            return t

        bq_bc = load_bias_bcast(bq_p)
        bk_bc = load_bias_bcast(bk_p)
        bp_bc = load_bias_bcast(bp_p)

        # bv as per-partition [128, 3] (v is produced channels-on-partitions)
        bv_sb = const.tile([128, 3], F32)
        nc.sync.dma_start(bv_sb[:], bv_p.ap().rearrange("(oc p) -> p oc", p=128))

        id48 = const.tile([48, 48], F32)
        make_identity(nc, id48[:])

        # attn block-diagonal matrices (rhs of stage 5): 3 chunks [128, C].
        # rows c = 48*h + dj, cols q2 = 8*di + h; off-block entries stay 0.
        bd = [const.tile([128, C], F32R, tag=f"bd{i}", name=f"bd{i}")
              for i in range(3)]
        for t in bd:
            nc.sync.dma_start(t[:], z_p.ap()[:, :])

        for b in range(BPC):
            # persistent per-batch accumulators
            vT = big.tile([128, 3, N], F32R, tag="vT")        # [c, n] channels/parts
            at = big.tile([128, C * NCHUNK], F32R, tag="at")  # AT[c', u]
            # u = 256*di + 32*h + t; stage-5 psum columns are (h, di) ordered
            atv = at[:].rearrange("p (d h t) -> p h d t", h=H, t=NCHUNK)
            atr = at[:].rearrange("p (r j) -> p r j", j=3)
            s_ps = ps_s.tile([48, H, 48], F32, tag="s")       # per-head S

            xb = xp.ap()[b].rearrange("(kc p) n -> p kc n", p=128)

            # ---- projections + S accumulation, streamed over N -------------
            for w in range(NWIN):
                xw = xin.tile([128, 3, NW], F32R, tag="xw")
                nc.sync.dma_start(xw[:], xb[:, :, w * NW:(w + 1) * NW])

                # vT window: [c(out) parts, n free]
                for oc in range(3):
                    v_ps = ps_big.tile([128, NW], F32, tag="psbig")
                    for kc in range(3):
                        nc.tensor.matmul(
                            v_ps[:],
                            wv_sb[:, kc, oc * 128:(oc + 1) * 128],
                            xw[:, kc, :],
                            start=(kc == 0), stop=(kc == 2),
                        )
                    nc.scalar.activation(
                        vT[:, oc, w * NW:(w + 1) * NW], v_ps[:],
                        mybir.ActivationFunctionType.Identity,
                        bias=bv_sb[:, oc:oc + 1], scale=1.0,
                    )

                # q/k windows: [n parts, c free], cast to bf16 for S matmuls
                for ns in range(NW // 128):
                    t_chunk = w * (NW // 128) + ns
                    nsl = slice(ns * 128, (ns + 1) * 128)
                    q_sb = qk.tile([128, C], BF16, tag="qsb")
                    k_sb = qk.tile([128, C], BF16, tag="ksb")
                    for dst, wsb, bbc in ((q_sb, wq_sb, bq_bc),
                                          (k_sb, wk_sb, bk_bc)):
                        p_ps = ps_qk.tile([128, C], F32, tag="psqk")
                        for kc in range(3):
                            nc.tensor.matmul(
                                p_ps[:],
                                xw[:, kc, nsl],
                                wsb[:, kc, :],
                                start=(kc == 0), stop=(kc == 2),
                            )
                        nc.vector.tensor_add(dst[:], p_ps[:], bbc[:])
                    # S accumulation, per head (output partitions 0..47)
                    for h in range(H):
                        csl = slice(h * 48, (h + 1) * 48)
                        nc.tensor.matmul(
                            s_ps[:, h, :], q_sb[:, csl], k_sb[:, csl],
                            start=(t_chunk == 0), stop=(t_chunk == NCHUNK - 1),
                        )

            # ---- softmax over dj (no max-subtraction: |S|/64 < 1) ----------
            p_all = sm.tile([48, H, 48], F32, tag="p_all")
            nc.scalar.activation(
                p_all[:], s_ps[:],
                mybir.ActivationFunctionType.Exp,
                bias=0.0, scale=SCALE,
            )
            zsum = sm.tile([48, H], F32, tag="zsum")
            nc.vector.reduce_sum(zsum[:], p_all[:], axis=mybir.AxisListType.X)
            zrec = sm.tile([48, H], F32, tag="zrec")
            nc.vector.reciprocal(zrec[:], zsum[:])
            attn = sm.tile([48, H, 48], F32, tag="attn")
            for h in range(H):
                nc.vector.tensor_scalar_mul(
                    attn[:, h, :], p_all[:, h, :], zrec[:, h:h + 1])

            # ---- transpose each head's attn and scatter into block-diag ----
            tr_ps = ps_tr.tile([48, H, 48], F32, tag="tr")
            for h in range(H):
                nc.tensor.transpose(tr_ps[:, h, :], attn[:, h, :], id48[:])
            attn_t = sm.tile([48, H, 48], F32R, tag="attn_t")
            nc.scalar.activation(
                attn_t[:], tr_ps[:],
                mybir.ActivationFunctionType.Identity, bias=0.0, scale=1.0,
            )
            # scatter attn_t[dj, h, di] -> bd[kc][48h+dj (mod 128), 48h:48h+48]
            # via DMA (engines can't address non-32-aligned partition bases)
            for h in range(H):
                c0 = 48 * h
                dj = 0
                while dj < 48:
                    kc, off = (c0 + dj) // 128, (c0 + dj) % 128
                    cnt = min(48 - dj, 128 - off)
                    nc.sync.dma_start(
                        bd[kc][off:off + cnt, c0:c0 + 48],
                        attn_t[dj:dj + cnt, h, :])
                    dj += cnt

            # ---- stage 5: AT[c', 32*q2 + t] = sum_c vT[c, n] * bd[c, q2] ---
            for t in range(NCHUNK):
                at_ps = ps_qk.tile([128, C], F32, tag="psqk")
                for kc in range(3):
                    nc.tensor.matmul(
                        at_ps[:],
                        vT[:, kc, t * 128:(t + 1) * 128],
                        bd[kc][:],
                        start=(kc == 0), stop=(kc == 2),
                    )
                nc.vector.tensor_copy(atv[:, :, :, t], at_ps[:])

            # ---- stage 6: Y[r, o] = sum_j AT[:, 3r+j]^T wpT[128j:, o] + bp -
            for rw in range(NCHUNK):
                y_ps = ps_qk.tile([128, C], F32, tag="psqk")
                for j in range(3):
                    nc.tensor.matmul(
                        y_ps[:],
                        atr[:, rw * 128:(rw + 1) * 128, j],
                        wp_sb[:, j, :],
                        start=(j == 0), stop=(j == 2),
                    )
                y_sb = yout.tile([128, C], F32, tag="ysb")
                nc.vector.tensor_add(y_sb[:], y_ps[:], bp_bc[:])
                nc.sync.dma_start(outp.ap()[b, rw * 128:(rw + 1) * 128, :], y_sb[:])

    nc.compile()
    return nc


_CACHE = {}


def kernel(x, conv_w, conv_b, wq, bq, wkv, bkv, wp, bp):
    _install_ntff_hook()
    x = np.ascontiguousarray(x, dtype=np.float32)

    # fold the 1x1 conv into the projections (host-side weight prep)
    wk_w, wv_w = wkv[:C], wkv[C:]
    bk_b, bv_b = bkv[:C], bkv[C:]
    f32 = np.float32
    wqT = np.ascontiguousarray((wq @ conv_w).T, dtype=f32)
    wkT = np.ascontiguousarray((wk_w @ conv_w).T, dtype=f32)
    wvT = np.ascontiguousarray((wv_w @ conv_w).T, dtype=f32)
    wpT = np.ascontiguousarray(wp.T, dtype=f32)
    bq_e = np.ascontiguousarray(bq + wq @ conv_b, dtype=f32)
    bk_e = np.ascontiguousarray(bk_b + wk_w @ conv_b, dtype=f32)
    bv_e = np.ascontiguousarray(bv_b + wv_w @ conv_b, dtype=f32)
    bp_c = np.ascontiguousarray(bp, dtype=f32)

    if "nc" not in _CACHE:
        _CACHE["nc"] = build()
    nc = _CACHE["nc"]

    xt = np.ascontiguousarray(x.transpose(0, 2, 1))  # [B, C, N]
    in_maps = []
    for c in range(N_CORES):
        in_maps.append({
            "x": xt[c * BPC:(c + 1) * BPC],
            "wqT": wqT, "wkT": wkT, "wvT": wvT, "wpT": wpT,
            "bq": bq_e, "bk": bk_e, "bv": bv_e, "bp": bp_c,
            "zeros": np.zeros((128, C), dtype=np.float32),
        })

    res = run_bass_kernel_spmd(nc, in_maps, core_ids=list(range(N_CORES)))
    out = np.concatenate([res.results[c]["out"] for c in range(N_CORES)], axis=0)
    return out.astype(np.float32)
